# revision 1
# baseline (speedup 1.0000x reference)
"""MGCN (3-layer RGCN-style message passing) on 8 Trainium2 NeuronCores.

Sharding: edges are sharded by destination-node range, aligned with a
node-range sharding of the output (core c owns nodes [c*NS, (c+1)*NS)).
Each core fully aggregates messages for its own nodes, so no all-reduce
is needed; an AllGather replicates the new node features between layers.

Per 128-node block, per 128-edge tile (edges sorted by dst):
  - indirect-DMA gather of source features Xg [128e, 128f]
  - DVE builds O_n[e,m] = (m == slot_e) * nw_e        (one dual-op instr)
        and O_w[e,(b,m)] = O_n[e,m] * att_e[b]        (one bcast instr)
  - one TensorE matmul accumulates Z^T[f,(b,m)] += Xg^T @ O_w in PSUM
A "self tile" gathers the block's own rows and multiplies by identity,
yielding x_blk^T for the root term. The epilogue applies the basis and
root matrices with 5 accumulating matmuls, adds bias (+ReLU on layer 2),
and writes the block's output rows.

Host side does index prep only: sort edges by dst, tile/pad, gather the
tiny att[edge_type] table rows, fold 1/deg into the edge norm, and cast
dtypes. All feature FLOPs run on device.
"""

import math

import numpy as np
import ml_dtypes

import concourse.bass as bass
import concourse.tile as tile
from concourse import bacc, mybir
from concourse.bass_utils import run_bass_kernel_spmd

P = 128
NBAS = 4
N_CORES = 8

BF16 = mybir.dt.bfloat16
F32 = mybir.dt.float32
I32 = mybir.dt.int32

_NP_OF = {BF16: ml_dtypes.bfloat16, F32: np.float32}


def build_program(n_cores, nblk, T, D=128, wdt=BF16):
    """Build the SPMD Bass program (same program for every core)."""
    NS = nblk * P
    NP_ = n_cores * NS
    nc = bacc.Bacc(num_devices=n_cores)
    Alu = mybir.AluOpType

    x0 = nc.declare_dram_parameter("x0", [NP_, D], wdt, isOutput=False)
    # offs column T holds the block's own node ids (self/root gather)
    offs = nc.declare_dram_parameter("offs", [nblk, T + 1, P], I32, isOutput=False)
    slot = nc.declare_dram_parameter("slot", [nblk, T, P], F32, isOutput=False)
    # attE* carry att[edge_type] * edge_norm / deg(dst), pre-folded on host
    attE1 = nc.declare_dram_parameter("attE1", [nblk, T, P, NBAS], wdt, isOutput=False)
    attE2 = nc.declare_dram_parameter("attE2", [nblk, T, P, NBAS], wdt, isOutput=False)
    basis1 = nc.declare_dram_parameter("basis1", [NBAS, D, D], wdt, isOutput=False)
    basis2 = nc.declare_dram_parameter("basis2", [NBAS, D, D], wdt, isOutput=False)
    root1 = nc.declare_dram_parameter("root1", [D, D], wdt, isOutput=False)
    root2 = nc.declare_dram_parameter("root2", [D, D], wdt, isOutput=False)
    biasT1 = nc.declare_dram_parameter("biasT1", [P, D], wdt, isOutput=False)
    biasT2 = nc.declare_dram_parameter("biasT2", [P, D], wdt, isOutput=False)
    iotaT = nc.declare_dram_parameter("iotaT", [P, P], F32, isOutput=False)
    identT = nc.declare_dram_parameter("identT", [P, P], wdt, isOutput=False)
    outp = nc.declare_dram_parameter("out", [NS, D], F32, isOutput=True)

    # (attE, param-set index, relu)
    layers = [(attE1, 0, False), (attE1, 0, True), (attE2, 1, False)]

    with tile.TileContext(nc) as tc:
        with (
            tc.tile_pool(name="const", bufs=1) as cp,
            tc.tile_pool(name="sb", bufs=4) as sb,
            tc.tile_pool(name="xgp", bufs=6) as xgp,
            tc.tile_pool(name="pp", bufs=2, space="PSUM") as pp,
            tc.tile_pool(name="dram", bufs=1, space="DRAM") as dp,
        ):
            iota_sb = cp.tile([P, P], F32, tag="iota")
            nc.sync.dma_start(iota_sb[:], iotaT[:])
            ident_sb = cp.tile([P, P], wdt, tag="ident")
            nc.sync.dma_start(ident_sb[:], identT[:])

            basis_sb = []
            root_sb = []
            bias_sb = []
            for i, (b_h, r_h, bi_h) in enumerate(
                ((basis1, root1, biasT1), (basis2, root2, biasT2))
            ):
                bt = cp.tile([P, NBAS, D], wdt, tag=f"basis{i}", name=f"basis_sb{i}")
                nc.sync.dma_start(bt[:], b_h[:].rearrange("b i o -> i b o"))
                basis_sb.append(bt)
                rt = cp.tile([P, D], wdt, tag=f"root{i}", name=f"root_sb{i}")
                nc.sync.dma_start(rt[:], r_h[:])
                root_sb.append(rt)
                bit = cp.tile([P, D], wdt, tag=f"bias{i}", name=f"bias_sb{i}")
                nc.sync.dma_start(bit[:], bi_h[:])
                bias_sb.append(bit)

            x_cur = x0
            for li, (attE, pi, relu) in enumerate(layers):
                last = li == len(layers) - 1
                if not last:
                    xs = dp.tile([NS, D], wdt, tag=f"xs{li}", name=f"xs{li}")
                    xnext = dp.tile(
                        [NP_, D], wdt, tag=f"xn{li}", name=f"xn{li}",
                        addr_space="Shared",
                    )
                for nb in range(nblk):
                    offs_sb = sb.tile(
                        [P, T + 1], I32, tag="offs", name=f"offs_{li}_{nb}"
                    )
                    nc.sync.dma_start(offs_sb[:], offs[nb].rearrange("t e -> e t"))
                    slot_sb = sb.tile([P, T], F32, tag="slot", name=f"slot_{li}_{nb}")
                    nc.sync.dma_start(slot_sb[:], slot[nb].rearrange("t e -> e t"))
                    attE_sb = sb.tile(
                        [P, T, NBAS], wdt, tag="attE", name=f"attE_{li}_{nb}"
                    )
                    nc.sync.dma_start(attE_sb[:], attE[nb].rearrange("t e b -> e t b"))

                    # all T one-hot slot matrices of the block in one DVE op
                    ona = sb.tile([P, T, P], wdt, tag="ona", name=f"ona_{li}_{nb}")
                    nc.vector.tensor_tensor(
                        out=ona[:],
                        in0=iota_sb[:, None, :].to_broadcast([P, T, P]),
                        in1=slot_sb[:, :, None].to_broadcast([P, T, P]),
                        op=Alu.is_equal,
                    )

                    zps = pp.tile([P, NBAS, P], F32, tag="z", name=f"z_{li}_{nb}")
                    for t in range(T):
                        xgt = xgp.tile([P, D], wdt, tag="xg", name=f"xg_{li}_{nb}_{t}")
                        nc.gpsimd.indirect_dma_start(
                            out=xgt[:],
                            out_offset=None,
                            in_=x_cur[:, :],
                            in_offset=bass.IndirectOffsetOnAxis(
                                ap=offs_sb[:, t : t + 1], axis=0
                            ),
                        )
                        xg = xgt[:]
                        ow = sb.tile(
                            [P, NBAS, P], wdt, tag="ow", name=f"ow_{li}_{nb}_{t}"
                        )
                        nc.vector.tensor_tensor(
                            out=ow[:],
                            in0=ona[:, t, :][:, None, :].to_broadcast([P, NBAS, P]),
                            in1=attE_sb[:, t, :][:, :, None].to_broadcast(
                                [P, NBAS, P]
                            ),
                            op=Alu.mult,
                        )
                        nc.tensor.matmul(
                            zps[:],
                            lhsT=xg,
                            rhs=ow[:],
                            start=(t == 0),
                            stop=(t == T - 1),
                        )
                    # self tile: x_blk^T via identity matmul (for the root term)
                    xgs = xgp.tile([P, D], wdt, tag="xgs", name=f"xgs_{li}_{nb}")
                    nc.gpsimd.indirect_dma_start(
                        out=xgs[:],
                        out_offset=None,
                        in_=x_cur[:, :],
                        in_offset=bass.IndirectOffsetOnAxis(
                            ap=offs_sb[:, T : T + 1], axis=0
                        ),
                    )
                    sps = pp.tile([P, P], F32, tag="s", name=f"s_{li}_{nb}")
                    nc.tensor.matmul(
                        sps[:], lhsT=xgs[:], rhs=ident_sb[:], start=True, stop=True
                    )

                    zt = sb.tile([P, NBAS, P], wdt, tag="zt", name=f"zt_{li}_{nb}")
                    nc.vector.tensor_copy(zt[:], zps[:])
                    xt = sb.tile([P, P], wdt, tag="xt", name=f"xt_{li}_{nb}")
                    nc.scalar.copy(xt[:], sps[:])

                    agg = pp.tile([P, P], F32, tag="agg", name=f"agg_{li}_{nb}")
                    for b in range(NBAS):
                        nc.tensor.matmul(
                            agg[:],
                            lhsT=zt[:, b, :],
                            rhs=basis_sb[pi][:, b, :],
                            start=(b == 0),
                            stop=False,
                        )
                    nc.tensor.matmul(
                        agg[:], lhsT=xt[:], rhs=root_sb[pi][:], start=False, stop=True
                    )

                    ob = sb.tile(
                        [P, D],
                        F32 if last else wdt,
                        tag="ob_f" if last else "ob",
                        name=f"ob_{li}_{nb}",
                    )
                    nc.vector.tensor_tensor(
                        out=ob[:], in0=agg[:], in1=bias_sb[pi][:], op=Alu.add
                    )
                    if relu:
                        nc.vector.tensor_scalar(
                            out=ob[:],
                            in0=ob[:],
                            scalar1=0.0,
                            scalar2=None,
                            op0=Alu.max,
                        )
                    dst_rows = outp if last else xs
                    nc.sync.dma_start(dst_rows[nb * P : (nb + 1) * P, :], ob[:])
                if not last:
                    nc.gpsimd.collective_compute(
                        "AllGather",
                        Alu.bypass,
                        replica_groups=[list(range(n_cores))],
                        ins=[xs[:]],
                        outs=[xnext[:]],
                    )
                    x_cur = xnext
    nc.compile()
    return nc


def prepare_inputs(
    entity, edge_index, edge_type, edge_norm, emb,
    att1, att2, basis1, basis2, root1, root2, bias1, bias2,
    n_cores=N_CORES, wdt=BF16,
):
    """Host-side index prep + sharding. Returns (in_maps, nblk, T, N, NS)."""
    npdt = _NP_OF[wdt]
    N = int(entity.shape[0])
    D = int(emb.shape[1])
    x_full = np.asarray(emb, np.float32)[np.asarray(entity, np.int64)]
    src = np.asarray(edge_index[0], np.int64)
    dst = np.asarray(edge_index[1], np.int64)
    et = np.asarray(edge_type, np.int64)
    norm = np.asarray(edge_norm, np.float32)

    NS = ((N + n_cores * P - 1) // (n_cores * P)) * P
    NP_ = NS * n_cores
    nblk = NS // P

    cnt = np.bincount(dst, minlength=NP_).astype(np.float32)
    nw_full = norm / np.maximum(cnt, 1.0)[dst]
    attE1_full = np.asarray(att1, np.float32)[et] * nw_full[:, None]
    attE2_full = np.asarray(att2, np.float32)[et] * nw_full[:, None]

    order = np.argsort(dst, kind="stable")
    gb_bounds = np.searchsorted(dst[order], np.arange(0, NP_ + 1, P))
    ecnt = np.diff(gb_bounds)
    T = max(1, int(math.ceil(ecnt.max() / P)))

    nGB = NP_ // P
    offs_a = np.zeros((nGB, T * P), np.int32)
    slot_a = np.zeros((nGB, T * P), np.float32)
    at1_a = np.zeros((nGB, T * P, NBAS), np.float32)
    at2_a = np.zeros((nGB, T * P, NBAS), np.float32)
    for gb in range(nGB):
        lo, hi = gb_bounds[gb], gb_bounds[gb + 1]
        k = hi - lo
        if k == 0:
            continue
        sel = order[lo:hi]
        offs_a[gb, :k] = src[sel]
        slot_a[gb, :k] = dst[sel] - gb * P
        at1_a[gb, :k] = attE1_full[sel]
        at2_a[gb, :k] = attE2_full[sel]

    x0 = np.zeros((NP_, D), np.float32)
    x0[:N] = x_full

    iotaT = np.tile(np.arange(P, dtype=np.float32), (P, 1))
    identT = np.eye(P, dtype=np.float32)

    common = {
        "basis1": np.asarray(basis1, np.float32).astype(npdt),
        "basis2": np.asarray(basis2, np.float32).astype(npdt),
        "root1": np.asarray(root1, np.float32).astype(npdt),
        "root2": np.asarray(root2, np.float32).astype(npdt),
        "biasT1": np.tile(np.asarray(bias1, np.float32)[None, :], (P, 1)).astype(npdt),
        "biasT2": np.tile(np.asarray(bias2, np.float32)[None, :], (P, 1)).astype(npdt),
        "iotaT": iotaT,
        "identT": identT.astype(npdt),
        "x0": x0.astype(npdt),
    }

    in_maps = []
    for c in range(n_cores):
        s = slice(c * nblk, (c + 1) * nblk)
        offs_c = np.concatenate(
            [
                offs_a[s].reshape(nblk, T, P),
                (c * NS + np.arange(NS, dtype=np.int32)).reshape(nblk, 1, P),
            ],
            axis=1,
        )
        in_maps.append(
            dict(
                common,
                offs=np.ascontiguousarray(offs_c),
                slot=slot_a[s].reshape(nblk, T, P),
                attE1=at1_a[s].reshape(nblk, T, P, NBAS).astype(npdt),
                attE2=at2_a[s].reshape(nblk, T, P, NBAS).astype(npdt),
            )
        )
    return in_maps, nblk, T, N, NS


_PROGRAM_CACHE = {}


def run(inputs_dict, n_cores=N_CORES, wdt=BF16, trace=False, trace_kwargs=None):
    """Full pipeline: prep, (cached) build, run, unshard. Returns (out, results)."""
    in_maps, nblk, T, N, NS = prepare_inputs(
        inputs_dict["entity"], inputs_dict["edge_index"], inputs_dict["edge_type"],
        inputs_dict["edge_norm"], inputs_dict["emb"],
        inputs_dict["att1"], inputs_dict["att2"],
        inputs_dict["basis1"], inputs_dict["basis2"],
        inputs_dict["root1"], inputs_dict["root2"],
        inputs_dict["bias1"], inputs_dict["bias2"],
        n_cores=n_cores, wdt=wdt,
    )
    key = (n_cores, nblk, T, wdt)
    if key not in _PROGRAM_CACHE:
        _PROGRAM_CACHE[key] = build_program(n_cores, nblk, T, wdt=wdt)
    nc = _PROGRAM_CACHE[key]
    kwargs = {}
    if trace:
        kwargs["trace"] = True
        if trace_kwargs:
            kwargs.update(trace_kwargs)
    res = run_bass_kernel_spmd(nc, in_maps, list(range(n_cores)), **kwargs)
    out = np.concatenate([res.results[c]["out"] for c in range(n_cores)], axis=0)[:N]
    return np.ascontiguousarray(out, dtype=np.float32), res


def kernel(**inputs):
    out, _ = run(inputs)
    return out



# revision 4
# speedup vs baseline: 1.5880x; 1.5880x over previous
"""MGCN (3-layer RGCN-style message passing) on 8 Trainium2 NeuronCores.

Sharding: nodes are RELABELED on the host into "tile-groups" of <=16 nodes
whose total in-degree is <=128; 8 groups make one 128-slot block, and blocks
are split evenly across the 8 cores (each core owns its blocks' output rows).
Every block therefore has an identical static structure: 8 edge tiles of
<=128 edges, tile t's destinations confined to slot band [16t, 16t+16).

Gathers use ONE `dma_gather` per block: node features are viewed as QUAD
rows (4 nodes / 1KB per row, table <= 32K rows so int16 indices fit); each
edge fetches its source's quad, and the sub-row is selected by 4 masked
matmuls per tile (host zeroes att coefficients of non-matching sub-slots,
so wrong sub-rows contribute exactly 0).

Per block, per layer:
  - ONE dma_gather of 1024 quad rows  Xq [128e, 8t, 512]
  - ONE DVE op builds the banded one-hot  ona[e,t,m16] = (iota16==slot')
    and ONE DVE op expands it to ow[e,t,(s,b),m16] = ona * attw_masked
  - per tile, 4 masked matmuls  zps[f, b, 16t:16t+16] += Xq_{t,s}^T @ ow_{t,s}
  - x_blk^T (own rows, previous layer's local output) arrives via a direct
    HWDGE transpose-DMA (root term), bias comes in as a 1-row matmul, then
    4 basis matmuls + root matmul accumulate agg[m, g]; the ACT engine does
    PSUM->SBUF copies and ReLU.
An AllGather replicates node features between layers.
"""

import numpy as np
import ml_dtypes

import concourse.bass as bass
import concourse.tile as tile
from concourse import bacc, mybir
from concourse.bass_utils import run_bass_kernel_spmd

P = 128
NBAS = 4
N_CORES = 8
NT = 8          # tiles (bands) per block
BW = P // NT    # band width in slots (16)
NIDX = NT * P   # gather indices per block
NIW = NIDX // 16

BF16 = mybir.dt.bfloat16
F32 = mybir.dt.float32
I16 = mybir.dt.int16

_NP_OF = {BF16: ml_dtypes.bfloat16, F32: np.float32}


def build_program(n_cores, nblk, D=128, wdt=BF16):
    """Build the SPMD Bass program (same program for every core)."""
    NS = nblk * P
    NP_ = n_cores * NS
    NQ = NP_ // 4
    QE = 4 * D
    nc = bacc.Bacc(num_devices=n_cores)
    Alu = mybir.AluOpType
    Act = mybir.ActivationFunctionType

    x0 = nc.declare_dram_parameter("x0", [NQ, QE], wdt, isOutput=False)
    x0loc = nc.declare_dram_parameter("x0loc", [NS, D], wdt, isOutput=False)
    widx = nc.declare_dram_parameter("widx", [nblk, P, NIW], I16, isOutput=False)
    # sa packs per-edge (slot', attw1 masked by sub-slot [16], attw2 [16])
    sa = nc.declare_dram_parameter("sa", [nblk, NT, P, 33], wdt, isOutput=False)
    basis1 = nc.declare_dram_parameter("basis1", [NBAS, D, D], wdt, isOutput=False)
    basis2 = nc.declare_dram_parameter("basis2", [NBAS, D, D], wdt, isOutput=False)
    root1 = nc.declare_dram_parameter("root1", [D, D], wdt, isOutput=False)
    root2 = nc.declare_dram_parameter("root2", [D, D], wdt, isOutput=False)
    bias1 = nc.declare_dram_parameter("bias1", [1, D], wdt, isOutput=False)
    bias2 = nc.declare_dram_parameter("bias2", [1, D], wdt, isOutput=False)
    iota16 = nc.declare_dram_parameter("iota16", [P, BW], wdt, isOutput=False)
    ones1 = nc.declare_dram_parameter("ones1", [1, P], wdt, isOutput=False)
    outp = nc.declare_dram_parameter("out", [NS, D], F32, isOutput=True)

    # (param-set index, relu)
    layers = [(0, False), (0, True), (1, False)]

    with tile.TileContext(nc) as tc:
        with (
            tc.tile_pool(name="const", bufs=1) as cp,
            tc.tile_pool(name="sb", bufs=4) as sb,
            tc.tile_pool(name="xgp", bufs=3) as xgp,
            tc.tile_pool(name="pp", bufs=2, space="PSUM") as pp,
            tc.tile_pool(name="dram", bufs=1, space="DRAM") as dp,
        ):
            iota_sb = cp.tile([P, BW], wdt, tag="iota")
            nc.sync.dma_start(iota_sb[:], iota16[:])
            ones_sb = cp.tile([1, P], wdt, tag="ones")
            nc.sync.dma_start(ones_sb[:], ones1[:])

            basis_sb = []
            root_sb = []
            bias_sb = []
            for i, (b_h, r_h, bi_h) in enumerate(
                ((basis1, root1, bias1), (basis2, root2, bias2))
            ):
                bt = cp.tile([P, NBAS, D], wdt, tag=f"basis{i}", name=f"basis_sb{i}")
                nc.sync.dma_start(bt[:], b_h[:].rearrange("b i o -> i b o"))
                basis_sb.append(bt)
                rt = cp.tile([P, D], wdt, tag=f"root{i}", name=f"root_sb{i}")
                nc.sync.dma_start(rt[:], r_h[:])
                root_sb.append(rt)
                bit = cp.tile([1, D], wdt, tag=f"bias{i}", name=f"bias_sb{i}")
                nc.sync.dma_start(bit[:], bi_h[:])
                bias_sb.append(bit)

            x_cur = x0          # quad view [NQ, QE] for gathers
            xt_src = x0loc      # own rows [NS, D] for the root term
            for li, (pi, relu) in enumerate(layers):
                last = li == len(layers) - 1
                if not last:
                    xs = dp.tile([NS, D], wdt, tag=f"xs{li}", name=f"xs{li}")
                    xnext = dp.tile(
                        [NQ, QE], wdt, tag=f"xn{li}", name=f"xn{li}",
                        addr_space="Shared",
                    )
                ac = 1 + 16 * pi  # att column range within sa
                for nb in range(nblk):
                    widx_sb = sb.tile([P, NIW], I16, tag="widx", name=f"wi_{li}_{nb}")
                    nc.sync.dma_start(widx_sb[:], widx[nb])
                    sa_sb = sb.tile([P, NT, 33], wdt, tag="sa", name=f"sa_{li}_{nb}")
                    nc.sync.dma_start(sa_sb[:], sa[nb].rearrange("t e c -> e t c"))

                    # one gather: every tile's source quad rows
                    xgq = xgp.tile([P, NT, QE], wdt, tag="xg", name=f"xg_{li}_{nb}")
                    nc.gpsimd.dma_gather(
                        out_ap=xgq[:],
                        in_ap=x_cur[:],
                        idxs_ap=widx_sb[:],
                        num_idxs=NIDX,
                        num_idxs_reg=NIDX,
                        elem_size=QE,
                    )
                    # block's own rows, transposed, for the root term
                    xt = sb.tile([P, P], wdt, tag="xt", name=f"xt_{li}_{nb}")
                    nc.sync.dma_start(
                        xt[:], xt_src[nb * P : (nb + 1) * P, :], transpose=True
                    )

                    # banded one-hot + masked-att expansion: 2 DVE ops
                    ona = sb.tile([P, NT, BW], wdt, tag="ona", name=f"on_{li}_{nb}")
                    nc.vector.tensor_tensor(
                        out=ona[:],
                        in0=iota_sb[:, None, :].to_broadcast([P, NT, BW]),
                        in1=sa_sb[:, :, 0][:, :, None].to_broadcast([P, NT, BW]),
                        op=Alu.is_equal,
                    )
                    ow = sb.tile([P, NT, 16, BW], wdt, tag="ow", name=f"ow_{li}_{nb}")
                    nc.vector.tensor_tensor(
                        out=ow[:],
                        in0=ona[:, :, None, :].to_broadcast([P, NT, 16, BW]),
                        in1=sa_sb[:, :, ac : ac + 16][:, :, :, None].to_broadcast(
                            [P, NT, 16, BW]
                        ),
                        op=Alu.mult,
                    )

                    zps = pp.tile([P, NBAS, P], F32, tag="z", name=f"z_{li}_{nb}")
                    for t in range(NT):
                        for s in range(4):
                            nc.tensor.matmul(
                                zps[:, :, t * BW : (t + 1) * BW],
                                lhsT=xgq[:, t, s * D : (s + 1) * D],
                                rhs=ow[:, t, 4 * s : 4 * s + 4, :],
                                start=(s == 0),
                                stop=(s == 3),
                            )
                    zt = sb.tile([P, NBAS, P], wdt, tag="zt", name=f"zt_{li}_{nb}")
                    nc.scalar.copy(zt[:], zps[:])

                    agg = pp.tile([P, P], F32, tag="agg", name=f"ag_{li}_{nb}")
                    nc.tensor.matmul(
                        agg[:], lhsT=ones_sb[:], rhs=bias_sb[pi][:],
                        start=True, stop=False,
                    )
                    for b in range(NBAS):
                        nc.tensor.matmul(
                            agg[:],
                            lhsT=zt[:, b, :],
                            rhs=basis_sb[pi][:, b, :],
                            start=False,
                            stop=False,
                        )
                    nc.tensor.matmul(
                        agg[:], lhsT=xt[:], rhs=root_sb[pi][:], start=False, stop=True
                    )

                    ob = sb.tile(
                        [P, D],
                        F32 if last else wdt,
                        tag="ob_f" if last else "ob",
                        name=f"ob_{li}_{nb}",
                    )
                    nc.scalar.activation(
                        ob[:], agg[:], Act.Relu if relu else Act.Copy
                    )
                    dst_rows = outp if last else xs
                    nc.sync.dma_start(dst_rows[nb * P : (nb + 1) * P, :], ob[:])
                if not last:
                    nc.gpsimd.collective_compute(
                        "AllGather",
                        mybir.AluOpType.bypass,
                        replica_groups=[list(range(n_cores))],
                        ins=[xs[:]],
                        outs=[xnext[:]],
                    )
                    x_cur = xnext
                    xt_src = xs
    nc.compile()
    return nc


def _pack_groups(deg, max_nodes=BW, max_edges=P):
    """First-fit sequential packing of nodes into tile-groups."""
    n = deg.shape[0]
    gid = np.empty(n, np.int64)
    gslot = np.empty(n, np.int64)
    g = 0
    cnt = 0
    esum = 0
    for i in range(n):
        d = deg[i]
        if cnt >= max_nodes or (cnt > 0 and esum + d > max_edges):
            g += 1
            cnt = 0
            esum = 0
        gid[i] = g
        gslot[i] = cnt
        cnt += 1
        esum += d
    return gid, gslot, g + 1


def prepare_inputs(
    entity, edge_index, edge_type, edge_norm, emb,
    att1, att2, basis1, basis2, root1, root2, bias1, bias2,
    n_cores=N_CORES, wdt=BF16,
):
    """Host-side packing + sharding. Returns (in_maps, nblk, newid, N)."""
    npdt = _NP_OF[wdt]
    N = int(entity.shape[0])
    D = int(emb.shape[1])
    x_full = np.asarray(emb, np.float32)[np.asarray(entity, np.int64)]
    src = np.asarray(edge_index[0], np.int64)
    dst = np.asarray(edge_index[1], np.int64)
    et = np.asarray(edge_type, np.int64)
    norm = np.asarray(edge_norm, np.float32)

    deg = np.bincount(dst, minlength=N)
    gid, gslot, ngroups = _pack_groups(deg)

    nblk = (ngroups + NT - 1) // NT
    nblk = (nblk + n_cores - 1) // n_cores
    NS = nblk * P
    NP_ = NS * n_cores
    NQ = NP_ // 4
    assert NQ <= 32767, f"quad table too large for int16: {NQ}"

    newid = gid * BW + gslot
    nsrc = newid[src]
    ndst = newid[dst]
    blk = ndst // P
    band = (ndst % P) // BW
    slotp = ndst % BW
    sub = nsrc % 4          # sub-slot within the quad row
    qidx = nsrc // 4

    nw = norm / np.maximum(deg, 1)[dst]
    attw1 = np.asarray(att1, np.float32)[et] * nw[:, None]
    attw2 = np.asarray(att2, np.float32)[et] * nw[:, None]

    key = (blk * NT + band) * BW + slotp
    order = np.argsort(key, kind="stable")
    bb = (blk * NT + band)[order]
    bounds = np.searchsorted(bb, np.arange(nblk * n_cores * NT + 1))

    nGBT = nblk * n_cores * NT
    idx_a = np.zeros((nGBT, P), np.int16)
    sa_a = np.zeros((nGBT, P, 33), np.float32)
    ecnt = np.diff(bounds)
    assert ecnt.max() <= P, f"band overflow: {ecnt.max()}"
    for gbt in np.nonzero(ecnt)[0]:
        lo, hi = bounds[gbt], bounds[gbt + 1]
        sel = order[lo:hi]
        k = hi - lo
        idx_a[gbt, :k] = qidx[sel]
        sa_a[gbt, :k, 0] = slotp[sel]
        # masked att: column 1 + s*4 + b is attw[b] iff sub == s
        s_sel = sub[sel]
        rows = np.arange(k)
        sa_a[gbt, rows[:, None], 1 + 4 * s_sel[:, None] + np.arange(4)[None, :]] = (
            attw1[sel]
        )
        sa_a[gbt, rows[:, None], 17 + 4 * s_sel[:, None] + np.arange(4)[None, :]] = (
            attw2[sel]
        )

    # wrap indices: flat position i -> widx[i % 16, i // 16], replicated x8
    idx_blocks = idx_a.reshape(n_cores * nblk, NIDX)
    widx_a = np.zeros((n_cores * nblk, P, NIW), np.int16)
    ii = np.arange(NIDX)
    widx_a[:, ii % 16, ii // 16] = idx_blocks
    for r in range(1, 8):
        widx_a[:, r * 16 : (r + 1) * 16, :] = widx_a[:, :16, :]

    x0 = np.zeros((NP_, D), np.float32)
    x0[newid] = x_full

    iota16 = np.tile(np.arange(BW, dtype=np.float32), (P, 1))

    common = {
        "basis1": np.asarray(basis1, np.float32).astype(npdt),
        "basis2": np.asarray(basis2, np.float32).astype(npdt),
        "root1": np.asarray(root1, np.float32).astype(npdt),
        "root2": np.asarray(root2, np.float32).astype(npdt),
        "bias1": np.asarray(bias1, np.float32)[None, :].astype(npdt),
        "bias2": np.asarray(bias2, np.float32)[None, :].astype(npdt),
        "iota16": iota16.astype(npdt),
        "ones1": np.ones((1, P), np.float32).astype(npdt),
        "x0": x0.reshape(NQ, 4 * D).astype(npdt),
    }

    sa_a = sa_a.reshape(n_cores * nblk, NT, P, 33)
    in_maps = []
    for c in range(n_cores):
        s = slice(c * nblk, (c + 1) * nblk)
        in_maps.append(
            dict(
                common,
                widx=np.ascontiguousarray(widx_a[s]),
                sa=np.ascontiguousarray(sa_a[s]).astype(npdt),
                x0loc=np.ascontiguousarray(
                    x0[c * NS : (c + 1) * NS]
                ).astype(npdt),
            )
        )
    return in_maps, nblk, newid, N


_PROGRAM_CACHE = {}


def run(inputs_dict, n_cores=N_CORES, wdt=BF16, trace=False, trace_kwargs=None):
    """Full pipeline: prep, (cached) build, run, unshard. Returns (out, results)."""
    in_maps, nblk, newid, N = prepare_inputs(
        inputs_dict["entity"], inputs_dict["edge_index"], inputs_dict["edge_type"],
        inputs_dict["edge_norm"], inputs_dict["emb"],
        inputs_dict["att1"], inputs_dict["att2"],
        inputs_dict["basis1"], inputs_dict["basis2"],
        inputs_dict["root1"], inputs_dict["root2"],
        inputs_dict["bias1"], inputs_dict["bias2"],
        n_cores=n_cores, wdt=wdt,
    )
    key = (n_cores, nblk, wdt)
    if key not in _PROGRAM_CACHE:
        _PROGRAM_CACHE[key] = build_program(n_cores, nblk, wdt=wdt)
    nc = _PROGRAM_CACHE[key]
    kwargs = {}
    if trace:
        kwargs["trace"] = True
        if trace_kwargs:
            kwargs.update(trace_kwargs)
    res = run_bass_kernel_spmd(nc, in_maps, list(range(n_cores)), **kwargs)
    full = np.concatenate([res.results[c]["out"] for c in range(n_cores)], axis=0)
    out = full[newid]
    return np.ascontiguousarray(out, dtype=np.float32), res


def kernel(**inputs):
    out, _ = run(inputs)
    return out
